# revision 61
# baseline (speedup 1.0000x reference)
"""DeepSeekV3 MLA attention prefill kernel for 8 Trainium2 NeuronCores.

Sharding: sequence-parallel for the low-rank input projections (q_a / kv_a),
AllGather of the shared latents, tensor-parallel over heads (4 heads/core) for
q_b / kv_b decompression and attention, AllGather of attention outputs, and
output-column-parallel o_proj (each core produces a disjoint 512-column slice
of the output, concatenated on host).

v2 restructure vs baseline:
  - all weights host-prearranged for contiguous whole-tensor DMA, resident in
    SBUF during phase 0 (no weight-stall on q_a)
  - kv_b decompression moved before q_b to overlap the q-latent AllGather
  - attention softmax chain software-pipelined (scores for kt+1 issued before
    the AV/den matmuls of kt) and all attention matmuls in bf16 (1 cycle/row)
  - reciprocal taken on the [1,512] denominator row before broadcast
  - o_proj interleaved with attention (block g runs after head g+1), PSUM
    accumulated into an SBUF fp32 accumulator (no DRAM read-modify-write),
    full-width g2 tile loads
"""

import sys

sys.path.insert(0, "/opt/trn_rl_repo")

import numpy as np
import ml_dtypes

import concourse.bass as bass  # noqa: F401
import concourse.mybir as mybir
from concourse import bacc
from concourse.bass import ds, ts
from concourse.tile import TileContext
from concourse.bass_utils import run_bass_kernel_spmd
from contextlib import ExitStack

F = mybir.dt.float32
BF = mybir.dt.bfloat16
R = mybir.dt.float32r
I32 = mybir.dt.int32
AF = mybir.ActivationFunctionType
ALU = mybir.AluOpType

NCORES = 8
B, S, H = 1, 2048, 4096
N_HEADS = 32
HPC = N_HEADS // NCORES          # heads per core = 4
SL = S // NCORES                 # sequence rows per core = 256
QR, KR = 1536, 512
DR, DN, DV = 64, 128, 128
QD = DN + DR                     # 192
SCALE = QD ** -0.5
EPS = 1e-6
THETA = 10000.0
TWO_PI = float(2.0 * np.pi)
MCOLS = H // NCORES              # output columns per core = 512

LAST_RESULT = None               # test harness reads exec_time_ns from here
_CACHED_NC = None
_UID = [0]


def _uid():
    _UID[0] += 1
    return _UID[0]


def _emit_range_reduce(nc, pool, t_ap, width):
    """In-place wrap t_ap (f32, [128, width]) to [-pi, pi]. f32->i32 copy
    rounds to nearest (verified on HW)."""
    tn = pool.tile([128, width], F, tag=f"rr_f_{width}", name=f"rrf{_uid()}")
    ti = pool.tile([128, width], I32, tag=f"rr_i_{width}", name=f"rri{_uid()}")
    nc.vector.tensor_scalar_mul(tn[:], t_ap, 1.0 / TWO_PI)
    nc.vector.tensor_copy(ti[:], tn[:])
    nc.vector.tensor_copy(tn[:], ti[:])
    nc.vector.tensor_scalar_mul(tn[:], tn[:], -TWO_PI)
    nc.vector.tensor_tensor(t_ap, t_ap, tn[:], ALU.add)


def _build_program():
    nc = bacc.Bacc(None, target_bir_lowering=False, num_devices=NCORES)

    # ---------------- DRAM declarations ----------------
    # host-prearranged: [128, kt, cols] layouts for contiguous DMA
    xr_d = nc.dram_tensor("xr", [128, 32 * SL], BF, kind="ExternalInput")
    kvawr_d = nc.dram_tensor("kvawr", [128, 32 * 576], BF, kind="ExternalInput")
    qawr_d = nc.dram_tensor("qawr", [128, 32 * QR], BF, kind="ExternalInput")
    qbwT = nc.dram_tensor("qbwT", [QR, 768], BF, kind="ExternalInput")
    kvbwT = nc.dram_tensor("kvbwT", [KR, 1024], BF, kind="ExternalInput")
    owr_d = nc.dram_tensor("owr", [128, 32 * MCOLS], BF, kind="ExternalInput")
    ident_d = nc.dram_tensor("ident", [128, 128], F, kind="ExternalInput")
    triu_d = nc.dram_tensor("triu", [128, 128], BF, kind="ExternalInput")
    sinq_d = nc.dram_tensor("sinq", [128, S], F, kind="ExternalInput")  # sign-folded
    cosq_d = nc.dram_tensor("cosq", [128, S], F, kind="ExternalInput")
    sink_d = nc.dram_tensor("sink", [128, 64], F, kind="ExternalInput")
    cosk_d = nc.dram_tensor("cosk", [128, 64], F, kind="ExternalInput")
    out_d = nc.dram_tensor("out", [MCOLS, S], BF, kind="ExternalOutput")

    g1kv_src = nc.dram_tensor("g1kv_src", [KR + DR, SL], BF)
    g1kv = nc.dram_tensor("g1kv", [NCORES, KR + DR, SL], BF, addr_space="Shared")
    g1q_src = nc.dram_tensor("g1q_src", [QR, SL], BF)
    g1q = nc.dram_tensor("g1q", [NCORES, QR, SL], BF, addr_space="Shared")
    g2_src = [nc.dram_tensor(f"g2src{h}", [DV, S], BF) for h in range(HPC)]
    g2 = [
        nc.dram_tensor(f"g2_{h}", [NCORES, DV, S], BF, addr_space="Shared")
        for h in range(HPC)
    ]
    g2_src3 = [nc.dram_tensor(f"g2src3{i}", [DV, S // 2], BF) for i in range(2)]
    g2h3 = [
        nc.dram_tensor(f"g2h3{i}", [NCORES, DV, S // 2], BF, addr_space="Shared")
        for i in range(2)
    ]
    RG = [list(range(NCORES))]
    NKT = H // 128  # 32 k-tiles over the model dim
    NR = QR // 128  # 12 k-tiles over q_lora_rank
    NKR = KR // 128  # 4 k-tiles over kv_lora_rank

    with TileContext(nc) as tc, ExitStack() as ctx:
        persist = ctx.enter_context(tc.tile_pool(name="persist", bufs=1))

        # ---------------- constants (DMAs issued after first x/w chunks) --
        ident = persist.tile([128, 128], F, name="c_ident")
        triu = persist.tile([128, 128], BF, name="c_triu")
        sin_k2 = persist.tile([128, 64], F, name="c_sink")
        cos_k2 = persist.tile([128, 64], F, name="c_cosk")
        ones_mat = persist.tile([128, 128], BF, name="c_ones_mat")
        nc.vector.memset(ones_mat[:], 1.0)
        ones_row = persist.tile([1, 128], BF, name="c_ones_row")
        nc.vector.memset(ones_row[:], 1.0)
        eps_t = persist.tile([128, 1], F, name="c_eps")
        nc.vector.memset(eps_t[:], EPS)
        identb = persist.tile([128, 128], BF, name="c_identb")
        sin_k = [sin_k2[:, ts(st, 32)] for st in range(2)]
        cos_k = [cos_k2[:, ts(st, 32)] for st in range(2)]

        # ---------------- phase 0: resident x + weights ----------------
        ctx0 = ExitStack()
        xtp = ctx0.enter_context(tc.tile_pool(name="xtp", bufs=1))
        p0 = ctx0.enter_context(tc.tile_pool(name="p0", bufs=2))

        xt = xtp.tile([128, NKT, SL], BF, name="xt")
        kvw = [
            xtp.tile([128, 8, 576], BF, tag=f"kvw{cg}", name=f"kvw{cg}")
            for cg in range(4)
        ]
        # interleave x / kv_a weight chunks so the first matmuls start early
        for cg in range(4):
            nc.sync.dma_start(
                xt[:, ds(8 * cg, 8)],
                xr_d.ap()[:, ds(cg * 8 * SL, 8 * SL)]
                .rearrange("p (kt s) -> p kt s", kt=8),
            )
            nc.sync.dma_start(
                kvw[cg][:],
                kvawr_d.ap()[:, ds(cg * 8 * 576, 8 * 576)]
                .rearrange("p (kt n) -> p kt n", kt=8),
            )
            if cg == 0:
                nc.sync.dma_start(ident[:], ident_d[:])
                nc.sync.dma_start(triu[:], triu_d[:])
                nc.sync.dma_start(sin_k2[:], sink_d[:])
                nc.sync.dma_start(cos_k2[:], cosk_d[:])
                nc.vector.tensor_copy(identb[:], ident[:])
        # q_a weights: 4 chunks of 8 k-tiles
        qw = []
        for cg in range(4):
            t = xtp.tile([128, 8, QR], BF, tag=f"qw{cg}", name=f"qw{cg}")
            nc.sync.dma_start(
                t[:],
                qawr_d.ap()[:, ds(cg * 8 * QR, 8 * QR)]
                .rearrange("p (kt n) -> p kt n", kt=8),
            )
            qw.append(t)

        # ---------------- kv_a projection ----------------
        ctx0a = ExitStack()
        kv_ps = ctx0a.enter_context(tc.tile_pool(name="kv_ps", bufs=4, space="PSUM"))
        kvch = [[None] * 2 for _ in range(2)]
        for ch in range(2):
            for st in range(2):
                kvch[st][ch] = kv_ps.tile(
                    [128, 288], F, tag="kv_ps", name=f"kvps{st}_{ch}"
                )
        for kt in range(NKT):
            w = kvw[kt // 8]
            for ch in range(2):
                for st in range(2):
                    nc.tensor.matmul(
                        kvch[st][ch][:], xt[:, kt, ts(st, 128)],
                        w[:, kt % 8, ts(ch, 288)],
                        start=(kt == 0), stop=(kt == NKT - 1),
                    )
        # copy kv PSUM to SBUF so the q_a PSUM pool can open immediately
        kvsb = [[None] * 2 for _ in range(2)]
        for st in range(2):
            for ch in range(2):
                t = p0.tile([128, 288], F, tag="kvsb", name=f"kvsb{st}_{ch}")
                nc.any.tensor_copy(t[:], kvch[st][ch][:])
                kvsb[st][ch] = t
        ctx0a.close()

        ctx0b = ExitStack()
        qa_ps = ctx0b.enter_context(tc.tile_pool(name="qa_ps", bufs=6, space="PSUM"))
        trp = ctx0b.enter_context(tc.tile_pool(name="trp", bufs=2, space="PSUM"))
        trsb = ctx0b.enter_context(tc.tile_pool(name="trsb", bufs=6))

        # kv tail compute (ACT/Vec only; runs while q_a st0 matmuls occupy PE)
        ckvns, kr_ts = [], []
        for st in range(2):
            acc0 = p0.tile([128, 1], F, tag="kvacc", name=f"kvacc0_{st}")
            acc1 = p0.tile([128, 1], F, tag="kvacc", name=f"kvacc1_{st}")
            scr = p0.tile([128, 288], F, tag="kvscr", name=f"kvscr{st}")
            nc.scalar.activation(scr[:], kvsb[st][0][:], AF.Square, accum_out=acc0[:])
            nc.scalar.activation(
                scr[:, 0:224], kvsb[st][1][:, 0:224], AF.Square, accum_out=acc1[:]
            )
            nc.vector.tensor_tensor(acc0[:], acc0[:], acc1[:], ALU.add)
            stdv = p0.tile([128, 1], F, tag="kvstd", name=f"kvstd{st}")
            nc.scalar.activation(stdv[:], acc0[:], AF.Sqrt, bias=eps_t[:], scale=1.0 / KR)
            rinv = p0.tile([128, 1], F, tag="kvrinv", name=f"kvrinv{st}")
            nc.vector.reciprocal(rinv[:], stdv[:])
            ckvn = p0.tile([128, KR], BF, tag="ckvn", name=f"ckvn{st}")
            nc.vector.tensor_scalar_mul(ckvn[:, 0:288], kvsb[st][0][:], rinv[:])
            nc.vector.tensor_scalar_mul(ckvn[:, 288:512], kvsb[st][1][:, 0:224], rinv[:])
            # rope k_pe: cols 512:576 of kv_a = chunk1 cols 224:288, deinterleaved
            pe = kvsb[st][1][:, 224:288].rearrange("p (d two) -> p two d", two=2)
            y1, y2 = pe[:, 0], pe[:, 1]
            kr_t = p0.tile([128, DR], BF, tag="kr", name=f"kr{st}")
            t1 = p0.tile([128, 32], F, tag="krt1", name=f"krt1_{st}")
            t2 = p0.tile([128, 32], F, tag="krt2", name=f"krt2_{st}")
            nc.vector.tensor_tensor(t1[:], y1, cos_k[st][:], ALU.mult)
            nc.vector.tensor_tensor(t2[:], y2, sin_k[st][:], ALU.mult)
            nc.vector.tensor_tensor(kr_t[:, 0:32], t1[:], t2[:], ALU.subtract)
            nc.vector.tensor_tensor(t1[:], y2, cos_k[st][:], ALU.mult)
            nc.vector.tensor_tensor(t2[:], y1, sin_k[st][:], ALU.mult)
            nc.vector.tensor_tensor(kr_t[:, 32:64], t1[:], t2[:], ALU.add)
            ckvns.append(ckvn)
            kr_ts.append(kr_t)

        qch = [[None] * 3 for _ in range(2)]
        for ch in range(3):
            for st in range(2):
                qch[st][ch] = qa_ps.tile(
                    [128, 512], F, tag="qa_ps", name=f"qaps{st}_{ch}"
                )

        def q_mms(st, kt_lo=0, kt_hi=NKT):
            for kt in range(kt_lo, kt_hi):
                w = qw[kt // 8]
                for ch in range(3):
                    nc.tensor.matmul(
                        qch[st][ch][:], xt[:, kt, ts(st, 128)],
                        w[:, kt % 8, ts(ch, 512)],
                        start=(kt == 0), stop=(kt == NKT - 1),
                    )

        def kv_tr(st):
            for rt in range(KR // 128):
                tp = trp.tile([128, 128], BF, tag="tr", name=f"kvtr{st}_{rt}")
                nc.tensor.transpose(tp[:], ckvns[st][:, ts(rt, 128)], identb[:])
                sb_t = trsb.tile([128, 128], BF, tag="trsb", name=f"kvtrs{st}_{rt}")
                nc.any.tensor_copy(sb_t[:], tp[:])
                nc.gpsimd.dma_start(g1kv_src.ap()[ts(rt, 128), ts(st, 128)], sb_t[:])
            tpk = trp.tile([64, 128], BF, tag="tr", name=f"kvtrk{st}")
            nc.tensor.transpose(tpk[:], kr_ts[st][:], identb[:])
            sb_k = trsb.tile([64, 128], BF, tag="trsb", name=f"kvtrks{st}")
            nc.any.tensor_copy(sb_k[:], tpk[:])
            nc.gpsimd.dma_start(g1kv_src.ap()[KR : KR + DR, ts(st, 128)], sb_k[:])

        # kv transposes interleaved into the q_a matmul stream so the PE never
        # waits on the transpose->copy->write chain (trp slot recycling)
        q_mms(0, 0, 8)  # ~7us of PE work: covers the kv rmsnorm latency
        kv_tr(0)
        q_mms(0, 8, 12)
        kv_tr(1)

        nc.gpsimd.collective_compute(
            "AllGather", ALU.bypass,
            ins=[g1kv_src.ap().opt()], outs=[g1kv.ap().opt()], replica_groups=RG,
        )

        def q_tail_rms(st):
            accs = []
            scr = p0.tile([128, 512], F, tag="qascr", name=f"qascr{st}")
            for ch in range(3):
                a = p0.tile([128, 1], F, tag="qaacc", name=f"qaacc{st}_{ch}")
                nc.scalar.activation(scr[:], qch[st][ch][:], AF.Square, accum_out=a[:])
                accs.append(a)
            nc.vector.tensor_tensor(accs[0][:], accs[0][:], accs[1][:], ALU.add)
            nc.vector.tensor_tensor(accs[0][:], accs[0][:], accs[2][:], ALU.add)
            stdv = p0.tile([128, 1], F, tag="qastd", name=f"qastd{st}")
            nc.scalar.activation(stdv[:], accs[0][:], AF.Sqrt, bias=eps_t[:], scale=1.0 / QR)
            rinv = p0.tile([128, 1], F, tag="qarinv", name=f"qarinv{st}")
            nc.vector.reciprocal(rinv[:], stdv[:])
            qn = p0.tile([128, QR], BF, tag="qn", name=f"qn{st}")
            for ch in range(3):
                nc.vector.tensor_scalar_mul(qn[:, ts(ch, 512)], qch[st][ch][:], rinv[:])
            return qn

        def q_tail_tr(st, qn, rt_lo, rt_hi):
            for rt in range(rt_lo, rt_hi):
                tp = trp.tile([128, 128], BF, tag="tr", name=f"qtr{st}_{rt}")
                nc.tensor.transpose(tp[:], qn[:, ts(rt, 128)], identb[:])
                sb_t = trsb.tile([128, 128], BF, tag="trsb", name=f"qtrs{st}_{rt}")
                nc.any.tensor_copy(sb_t[:], tp[:])
                nc.gpsimd.dma_start(g1q_src.ap()[ts(rt, 128), ts(st, 128)], sb_t[:])

        q_mms(0, 12, NKT)
        qn0 = q_tail_rms(0)  # ACT/Vec: overlaps st1 matmuls
        q_mms(1, 0, 20)
        q_tail_tr(0, qn0, 0, 6)
        q_mms(1, 20, 26)
        q_tail_tr(0, qn0, 6, NR)
        q_mms(1, 26, NKT)
        qn1 = q_tail_rms(1)
        q_tail_tr(1, qn1, 0, NR)
        ctx0b.close()
        ctx0.close()  # frees x + weights SBUF

        nc.gpsimd.collective_compute(
            "AllGather", ALU.bypass,
            ins=[g1q_src.ap().opt()], outs=[g1q.ap().opt()], replica_groups=RG,
        )

        # ---------------- shared latents on-chip ----------------
        ctx_att = ExitStack()
        attb = ctx_att.enter_context(tc.tile_pool(name="attb", bufs=1))

        kpe_rep = attb.tile([128, S], BF, name="kpe_rep")
        for half in range(2):
            nc.sync.dma_start(
                kpe_rep[ds(64 * half, 64), :].rearrange("p (c s) -> p c s", c=NCORES),
                g1kv.ap()[:, KR : KR + DR, :].rearrange("c p s -> p c s"),
            )

        qnope = [attb.tile([128, S], BF, name=f"qnope{h}") for h in range(HPC)]
        qfpe = [attb.tile([128, S], BF, name=f"qfpe{p}") for p in range(2)]
        v_sb = attb.tile([128, S // 128, 512], BF, name="v_sb")
        kn_all = [attb.tile([128, S], BF, name=f"kn{h}") for h in range(HPC)]

        # q rope tables from host (sin_q/cos_q over full S)
        ctx_sin = ExitStack()  # sin_q/cos_q live until after q_b
        sinp = ctx_sin.enter_context(tc.tile_pool(name="sinp", bufs=1))
        sin_q = sinp.tile([128, S], F, name="t_sinq")
        nc.sync.dma_start(sin_q[:], sinq_d[:])
        cos_q = sinp.tile([128, S], F, name="t_cosq")
        nc.sync.dma_start(cos_q[:], cosq_d[:])

        # ---------------- kv_b decompression (before q_b: overlaps AG2) ----
        ctxd = ExitStack()
        ckvp = ctxd.enter_context(tc.tile_pool(name="ckvp", bufs=1))
        dec_ps = ctxd.enter_context(tc.tile_pool(name="dec_ps", bufs=2, space="PSUM"))
        ckv_t = []
        for r in range(NKR):
            t = ckvp.tile([128, S], BF, name=f"ckv{r}")
            nc.sync.dma_start(
                t[:].rearrange("p (c s) -> p c s", c=NCORES),
                g1kv.ap()[:, ts(r, 128), :].rearrange("c p s -> p c s"),
            )
            ckv_t.append(t)
        kvbv, kvbn = [], []
        for r in range(NKR):
            tv = ckvp.tile([128, 512], BF, name=f"kvbv{r}")
            nc.sync.dma_start(tv[:], kvbwT.ap()[ts(r, 128), 512:1024])
            kvbv.append(tv)
            tn = ckvp.tile([128, 512], BF, name=f"kvbn{r}")
            nc.sync.dma_start(tn[:], kvbwT.ap()[ts(r, 128), 0:512])
            kvbn.append(tn)

        for st in range(S // 128):
            ps = dec_ps.tile([128, 512], F, tag="dec", name=f"vps{st}")
            for r in range(NKR):
                nc.tensor.matmul(
                    ps[:], ckv_t[r][:, ts(st, 128)], kvbv[r][:],
                    start=(r == 0), stop=(r == NKR - 1),
                )
            nc.any.tensor_copy(v_sb[:, st, :], ps[:])
        for h in range(HPC):
            for sb in range(4):
                ps = dec_ps.tile([128, 512], F, tag="dec", name=f"knps{h}_{sb}")
                for r in range(NKR):
                    nc.tensor.matmul(
                        ps[:], kvbn[r][:, ts(h, 128)], ckv_t[r][:, ts(sb, 512)],
                        start=(r == 0), stop=(r == NKR - 1),
                    )
                nc.any.tensor_copy(kn_all[h][:, ts(sb, 512)], ps[:])
        ctxd.close()

        # ---------------- q_b projection (+ q rope) ----------------
        ctx2 = ExitStack()
        qrp = ctx2.enter_context(tc.tile_pool(name="qrp", bufs=24))
        qbwp = ctx2.enter_context(tc.tile_pool(name="qbwp", bufs=1))
        ropep = ctx2.enter_context(tc.tile_pool(name="ropep", bufs=2))
        qb_ps = ctx2.enter_context(tc.tile_pool(name="qb_ps", bufs=6, space="PSUM"))

        qbw_all = qbwp.tile([128, NR, 768], BF, name="qbw_all")
        nc.sync.dma_start(
            qbw_all[:], qbwT.ap().rearrange("(r p) n -> p r n", p=128)
        )
        for sb in range(4):
            qr_tiles = []
            for r in range(NR):
                t = qrp.tile([128, 512], BF, tag="qr", name=f"qr{sb}_{r}")
                nc.sync.dma_start(
                    t[:].rearrange("p (c s) -> p c s", c=2),
                    g1q.ap()[2 * sb : 2 * sb + 2, ts(r, 128), :]
                    .rearrange("c p s -> p c s"),
                )
                qr_tiles.append(t)
            psj = [
                qb_ps.tile([128, 512], F, tag="qb_ps", name=f"qbps{sb}_{j}")
                for j in range(6)
            ]
            for r in range(NR):
                for j in range(6):
                    nc.tensor.matmul(
                        psj[j][:], qbw_all[:, r, ts(j, 128)], qr_tiles[r][:],
                        start=(r == 0), stop=(r == NR - 1),
                    )
            for h in range(HPC):
                nc.any.tensor_copy(qnope[h][:, ts(sb, 512)], psj[h][:])
            for p in range(2):
                t1 = ropep.tile([128, 512], F, tag="rope1", name=f"rp1_{sb}_{p}")
                t2 = ropep.tile([128, 512], F, tag="rope2", name=f"rp2_{sb}_{p}")
                nc.vector.tensor_tensor(
                    t1[:], psj[4 + p][:], cos_q[:, ts(sb, 512)], ALU.mult
                )
                for strip in range(4):
                    s0 = 32 * strip
                    nc.vector.tensor_tensor(
                        t2[ds(s0, 32), :],
                        psj[4 + p][ds(s0 ^ 32, 32), :],
                        sin_q[ds(s0, 32), ts(sb, 512)],
                        ALU.mult,
                    )
                nc.vector.tensor_tensor(qfpe[p][:, ts(sb, 512)], t1[:], t2[:], ALU.add)
        ctx2.close()
        ctx_sin.close()

        # ---------------- attention + interleaved o_proj ----------------
        ctx3 = ExitStack()
        probp = ctx3.enter_context(tc.tile_pool(name="probp", bufs=5))
        attp = ctx3.enter_context(tc.tile_pool(name="attp", bufs=3))
        denp = ctx3.enter_context(tc.tile_pool(name="denp", bufs=2))
        sc_ps = ctx3.enter_context(tc.tile_pool(name="sc_ps", bufs=3, space="PSUM"))
        av_ps = ctx3.enter_context(tc.tile_pool(name="av_ps", bufs=2, space="PSUM"))
        den_ps = ctx3.enter_context(tc.tile_pool(name="den_ps", bufs=2, space="PSUM"))
        bc_ps = ctx3.enter_context(tc.tile_pool(name="bc_ps", bufs=1, space="PSUM"))

        # o_proj resources (interleaved with attention)
        owp = ctx3.enter_context(tc.tile_pool(name="owp", bufs=1))
        g2p = ctx3.enter_context(tc.tile_pool(name="g2p", bufs=10))
        osbp = ctx3.enter_context(tc.tile_pool(name="osbp", bufs=1))
        ow_all = owp.tile([128, 32, MCOLS], BF, name="ow_all")
        nc.sync.dma_start(
            ow_all[:], owr_d.ap().rearrange("p (b m) -> p b m", b=32)
        )
        osb = [
            [osbp.tile([128, 512], F, name=f"osb{mt}_{sblk}") for sblk in range(4)]
            for mt in range(4)
        ]

        def oproj_block(g, half=None):
            g2rows = []
            for r8 in range(NCORES):
                if g == HPC - 1:
                    gt = g2p.tile(
                        [128, S // 2], BF, tag="g2th", name=f"g2t{g}_{half}_{r8}",
                        bufs=8,
                    )
                    nc.sync.dma_start(gt[:], g2h3[half].ap()[r8, :, :])
                else:
                    gt = g2p.tile([128, S], BF, tag="g2t", name=f"g2t{g}_{r8}")
                    nc.sync.dma_start(gt[:], g2[g].ap()[r8, :, :])
                g2rows.append(gt)
            sblks = range(4) if g < HPC - 1 else range(2 * half, 2 * half + 2)
            for sblk in sblks:
                loc = sblk if g < HPC - 1 else sblk - 2 * half
                for mt in range(4):
                    ps = sc_ps.tile([128, 512], F, tag="sc", name=f"ops{g}_{sblk}_{mt}")
                    for r8 in range(NCORES):
                        nc.tensor.matmul(
                            ps[:], ow_all[:, HPC * r8 + g, ts(mt, 128)],
                            g2rows[r8][:, ts(loc, 512)],
                            start=(r8 == 0), stop=(r8 == NCORES - 1),
                        )
                    if g == 0:
                        nc.any.tensor_copy(osb[mt][sblk][:], ps[:])
                    elif g < HPC - 1:
                        nc.vector.tensor_tensor(
                            osb[mt][sblk][:], osb[mt][sblk][:], ps[:], ALU.add
                        )
                    else:
                        obf = attp.tile(
                            [128, 512], BF, tag="obf", name=f"obf{mt}_{sblk}"
                        )
                        nc.vector.tensor_tensor(
                            obf[:], osb[mt][sblk][:], ps[:], ALU.add
                        )
                        nc.gpsimd.dma_start(
                            out_d.ap()[ts(mt, 128), ts(sblk, 512)], obf[:]
                        )

        # deferred softmax normalize: bc matmul + mult emitted after the next
        # qb's first score matmuls so the PE never waits on the Vec reciprocal
        pending_norm = [None]

        def flush_norm():
            if pending_norm[0] is None:
                return
            avt_p, den_sb_p, h_p, qb_p = pending_norm[0]
            pending_norm[0] = None
            bc = bc_ps.tile([128, 512], F, tag="bc", name=f"bc{h_p}_{qb_p}")
            nc.tensor.matmul(bc[:], ones_row[:], den_sb_p[:], start=True, stop=True)
            bcs = denp.tile([128, 512], BF, tag="bcs", name=f"bcs{h_p}_{qb_p}")
            nc.scalar.activation(bcs[:], bc[:], AF.Copy)
            att = attp.tile([128, 512], BF, tag="att", name=f"att{h_p}_{qb_p}")
            nc.vector.tensor_tensor(att[:], avt_p[:], bcs[:], ALU.mult)
            if h_p == HPC - 1:
                nc.gpsimd.dma_start(
                    g2_src3[qb_p // 2].ap()[:, ts(qb_p % 2, 512)], att[:]
                )
            else:
                nc.gpsimd.dma_start(g2_src[h_p].ap()[:, ts(qb_p, 512)], att[:])

        for h in range(HPC):
            kn = kn_all[h]
            pe_rhs = qfpe[h // 2][ds(64 * (h % 2), 64), :]
            pe_lhs = kpe_rep[ds(64 * (h % 2), 64), :]
            for qb in range(4):
                nkt = 4 * (qb + 1)
                avt = av_ps.tile([128, 512], F, tag="av", name=f"av{h}_{qb}")
                dent = den_ps.tile([128, 512], F, tag="den", name=f"den{h}_{qb}")

                def av_den(pprob, pkt, ptrim, last):
                    nc.tensor.matmul(
                        avt[:, ptrim:512], v_sb[:, pkt, ts(h, 128)],
                        pprob[:, ptrim:512],
                        start=(pkt == 0), stop=last,
                    )
                    nc.tensor.matmul(
                        dent[:, ptrim:512], ones_mat[:], pprob[:, ptrim:512],
                        start=(pkt == 0), stop=last,
                    )

                pend = []  # (prob, kt, trim), depth-2 software pipeline
                for kt in range(nkt):
                    trim = max(0, 128 * (kt - 4 * qb))
                    qsl = ds(512 * qb + trim, 512 - trim)
                    sct = sc_ps.tile([128, 512], F, tag="sc", name=f"sc{h}{qb}_{kt}")
                    nc.tensor.matmul(
                        sct[:, trim:512], kn[:, ts(kt, 128)], qnope[h][:, qsl],
                        start=True, stop=False,
                    )
                    nc.tensor.matmul(
                        sct[:, trim:512], pe_lhs[:, ts(kt, 128)], pe_rhs[:, qsl],
                        start=False, stop=True,
                    )
                    if kt == 2:
                        flush_norm()
                    if len(pend) == 2:
                        av_den(*pend.pop(0), last=False)
                    prob = probp.tile([128, 512], BF, tag="prob", name=f"pr{h}{qb}_{kt}")
                    nc.scalar.activation(prob[:, trim:512], sct[:, trim:512], AF.Exp)
                    if kt >= 4 * qb:
                        nc.vector.tensor_tensor(
                            prob[:, trim : trim + 128],
                            prob[:, trim : trim + 128],
                            triu[:],
                            ALU.mult,
                        )
                    pend.append((prob, kt, trim))
                while pend:
                    av_den(*pend.pop(0), last=(len(pend) == 0))
                # 1/den via bit-trick seed + one Newton step on DVE (the DVE
                # RECIPROCAL op has a ~3us fixed cost; ACT Ln would thrash
                # activation tables against Exp). |rel err| < 0.1% + bf16.
                y0i = denp.tile([1, 512], I32, tag="y0i", name=f"y0i{h}_{qb}")
                nc.vector.tensor_scalar(
                    y0i[:], dent[0:1, :].bitcast(I32), -1, None, ALU.bitwise_xor
                )
                nc.vector.tensor_scalar(y0i[:], y0i[:], 0x7EF311C4, None, ALU.add)
                nt = denp.tile([1, 512], F, tag="nt", name=f"nt{h}_{qb}")
                nc.vector.tensor_tensor(
                    nt[:], dent[0:1, :], y0i[:].bitcast(F), ALU.mult
                )
                nc.vector.tensor_scalar(nt[:], nt[:], -1.0, 2.0, ALU.mult, ALU.add)
                den_sb = denp.tile([1, 512], BF, tag="densb", name=f"densb{h}_{qb}")
                nc.vector.tensor_tensor(
                    den_sb[:], y0i[:].bitcast(F), nt[:], ALU.mult
                )
                pending_norm[0] = (avt, den_sb, h, qb)
                if h == HPC - 1 and qb == 1:
                    # first half of the last head gathers early so its o_proj
                    # half can run as soon as the head finishes
                    flush_norm()
                    nc.gpsimd.collective_compute(
                        "AllGather", ALU.bypass,
                        ins=[g2_src3[0].ap().opt()], outs=[g2h3[0].ap().opt()],
                        replica_groups=RG,
                    )

            flush_norm()  # last qb's normalize must land before the AllGather
            if h < HPC - 1:
                nc.gpsimd.collective_compute(
                    "AllGather", ALU.bypass,
                    ins=[g2_src[h].ap().opt()], outs=[g2[h].ap().opt()],
                    replica_groups=RG,
                )
                if h > 0:
                    oproj_block(h - 1)
            else:
                nc.gpsimd.collective_compute(
                    "AllGather", ALU.bypass,
                    ins=[g2_src3[1].ap().opt()], outs=[g2h3[1].ap().opt()],
                    replica_groups=RG,
                )
                oproj_block(HPC - 2)
                oproj_block(HPC - 1, half=0)
                oproj_block(HPC - 1, half=1)
        ctx3.close()
        ctx_att.close()

    nc.compile()
    return nc


def _get_nc():
    global _CACHED_NC
    if _CACHED_NC is None:
        _CACHED_NC = _build_program()
    return _CACHED_NC


def _prep_inputs(hidden_states, position_ids, q_a_w, q_a_ln_w, q_b_w, kv_a_w,
                 kv_a_ln_w, kv_b_w, o_w):
    hidden_states = np.asarray(hidden_states, dtype=np.float32)
    position_ids = np.asarray(position_ids, dtype=np.int32)
    q_a_w = np.asarray(q_a_w, dtype=np.float32)
    q_a_ln_w = np.asarray(q_a_ln_w, dtype=np.float32)
    q_b_w = np.asarray(q_b_w, dtype=np.float32)
    kv_a_w = np.asarray(kv_a_w, dtype=np.float32)
    kv_a_ln_w = np.asarray(kv_a_ln_w, dtype=np.float32)
    kv_b_w = np.asarray(kv_b_w, dtype=np.float32)
    o_w = np.asarray(o_w, dtype=np.float32)

    bf = ml_dtypes.bfloat16
    xT_full = np.ascontiguousarray(hidden_states[0].T)          # (H, S)
    # [H=(kt p), n] -> [p, kt*n] prearrangement for contiguous DMA
    qawT = q_a_w.T.reshape(32, 128, QR).transpose(1, 0, 2).reshape(128, 32 * QR)
    kvawT = kv_a_w.T.reshape(32, 128, 576).transpose(1, 0, 2).reshape(128, 32 * 576)
    ident = np.eye(128, dtype=np.float32)
    triu = np.triu(np.ones((128, 128), dtype=np.float32))        # keep k <= q
    inv_freq = (1.0 / (THETA ** (np.arange(0, DR, 2) / DR))).astype(np.float32)
    if128 = np.tile(inv_freq, 4)                                 # (128,)
    posf = position_ids.reshape(-1).astype(np.float64)           # (S,)
    ang_q = if128[:, None] * posf[None, :]                       # (128, S)
    # sign-folded sin for the strip-shifted rope:
    # qfpe[d] = pe_d[d]*cos[d] + pe_d[d^32]*sgn[d]*sin[d], sgn=-1 for d%64<32
    sgn = np.where((np.arange(128) % 64) < 32, -1.0, 1.0)[:, None]
    sinq = (sgn * np.sin(ang_q)).astype(np.float32)
    cosq = np.cos(ang_q).astype(np.float32)
    owT_full = np.ascontiguousarray(o_w.T)                       # (N*DV, H)

    in_maps = []
    for c in range(NCORES):
        heads = slice(HPC * c, HPC * (c + 1))
        qb = q_b_w.reshape(N_HEADS, QD, QR)[heads]               # (4, 192, QR)
        nope = qb[:, :DN, :].reshape(HPC * DN, QR)
        pe = qb[:, DN:, :]
        pe_d = np.concatenate([pe[:, 0::2, :], pe[:, 1::2, :]], axis=1)  # (4,64,QR)
        cols = np.concatenate(
            [nope, pe_d.reshape(HPC * DR, QR)], axis=0
        )                                                        # (768, QR)
        qbwT_c = np.ascontiguousarray((cols * (SCALE * q_a_ln_w[None, :])).T)

        kvb = kv_b_w.reshape(N_HEADS, DN + DV, KR)[heads]
        kcols = np.concatenate(
            [kvb[:, :DN, :].reshape(HPC * DN, KR),
             kvb[:, DN:, :].reshape(HPC * DV, KR)],
            axis=0,
        )                                                        # (1024, KR)
        kvbwT_c = np.ascontiguousarray((kcols * kv_a_ln_w[None, :]).T)

        ow_slice = owT_full[:, MCOLS * c : MCOLS * (c + 1)]      # (4096, 512)
        owr = ow_slice.reshape(32, 128, MCOLS).transpose(1, 0, 2).reshape(
            128, 32 * MCOLS
        )

        x_slice = xT_full[:, SL * c : SL * (c + 1)]              # (H, SL)
        xr = x_slice.reshape(32, 128, SL).transpose(1, 0, 2).reshape(128, 32 * SL)

        pos_loc = posf[SL * c : SL * (c + 1)]                    # (256,)
        ang_k = pos_loc.reshape(2, 128).T[:, :, None] * inv_freq[None, None, :]
        sink = np.sin(ang_k).reshape(128, 64).astype(np.float32)  # (128, 2*32)
        cosk = np.cos(ang_k).reshape(128, 64).astype(np.float32)

        in_maps.append(
            {
                "xr": np.ascontiguousarray(xr).astype(bf),
                "qawr": np.ascontiguousarray(qawT).astype(bf),
                "kvawr": np.ascontiguousarray(kvawT).astype(bf),
                "qbwT": qbwT_c.astype(bf),
                "kvbwT": kvbwT_c.astype(bf),
                "owr": np.ascontiguousarray(owr).astype(bf),
                "ident": ident,
                "triu": triu.astype(bf),
                "sinq": sinq,
                "cosq": cosq,
                "sink": np.ascontiguousarray(sink),
                "cosk": np.ascontiguousarray(cosk),
            }
        )
    return in_maps


def kernel(**inputs):
    global LAST_RESULT
    nc = _get_nc()
    in_maps = _prep_inputs(**inputs)
    res = run_bass_kernel_spmd(nc, in_maps, list(range(NCORES)))
    LAST_RESULT = res
    out = np.concatenate([res.results[c]["out"].T for c in range(NCORES)], axis=1)
    return out[None].astype(np.float32)


# revision 62
# speedup vs baseline: 1.0509x; 1.0509x over previous
"""DeepSeekV3 MLA attention prefill kernel for 8 Trainium2 NeuronCores.

Sharding: sequence-parallel for the low-rank input projections (q_a / kv_a),
AllGather of the shared latents, tensor-parallel over heads (4 heads/core) for
q_b / kv_b decompression and attention, AllGather of attention outputs, and
output-column-parallel o_proj (each core produces a disjoint 512-column slice
of the output, concatenated on host).

v2 restructure vs baseline:
  - all weights host-prearranged for contiguous whole-tensor DMA, resident in
    SBUF during phase 0 (no weight-stall on q_a)
  - kv_b decompression moved before q_b to overlap the q-latent AllGather
  - attention softmax chain software-pipelined (scores for kt+1 issued before
    the AV/den matmuls of kt) and all attention matmuls in bf16 (1 cycle/row)
  - reciprocal taken on the [1,512] denominator row before broadcast
  - o_proj interleaved with attention (block g runs after head g+1), PSUM
    accumulated into an SBUF fp32 accumulator (no DRAM read-modify-write),
    full-width g2 tile loads
"""

import sys

sys.path.insert(0, "/opt/trn_rl_repo")

import numpy as np
import ml_dtypes

import concourse.bass as bass  # noqa: F401
import concourse.mybir as mybir
from concourse import bacc
from concourse.bass import ds, ts
from concourse.tile import TileContext
from concourse.bass_utils import run_bass_kernel_spmd
from contextlib import ExitStack

F = mybir.dt.float32
BF = mybir.dt.bfloat16
R = mybir.dt.float32r
I32 = mybir.dt.int32
AF = mybir.ActivationFunctionType
ALU = mybir.AluOpType

NCORES = 8
B, S, H = 1, 2048, 4096
N_HEADS = 32
HPC = N_HEADS // NCORES          # heads per core = 4
SL = S // NCORES                 # sequence rows per core = 256
QR, KR = 1536, 512
DR, DN, DV = 64, 128, 128
QD = DN + DR                     # 192
SCALE = QD ** -0.5
EPS = 1e-6
THETA = 10000.0
TWO_PI = float(2.0 * np.pi)
MCOLS = H // NCORES              # output columns per core = 512

LAST_RESULT = None               # test harness reads exec_time_ns from here
_CACHED_NC = None
_UID = [0]


def _uid():
    _UID[0] += 1
    return _UID[0]


def _emit_range_reduce(nc, pool, t_ap, width):
    """In-place wrap t_ap (f32, [128, width]) to [-pi, pi]. f32->i32 copy
    rounds to nearest (verified on HW)."""
    tn = pool.tile([128, width], F, tag=f"rr_f_{width}", name=f"rrf{_uid()}")
    ti = pool.tile([128, width], I32, tag=f"rr_i_{width}", name=f"rri{_uid()}")
    nc.vector.tensor_scalar_mul(tn[:], t_ap, 1.0 / TWO_PI)
    nc.vector.tensor_copy(ti[:], tn[:])
    nc.vector.tensor_copy(tn[:], ti[:])
    nc.vector.tensor_scalar_mul(tn[:], tn[:], -TWO_PI)
    nc.vector.tensor_tensor(t_ap, t_ap, tn[:], ALU.add)


def _build_program():
    nc = bacc.Bacc(None, target_bir_lowering=False, num_devices=NCORES)

    # ---------------- DRAM declarations ----------------
    # host-prearranged: [128, kt, cols] layouts for contiguous DMA
    xr_d = nc.dram_tensor("xr", [128, 32 * SL], BF, kind="ExternalInput")
    kvawr_d = nc.dram_tensor("kvawr", [128, 32 * 576], BF, kind="ExternalInput")
    qawr_d = nc.dram_tensor("qawr", [128, 32 * QR], BF, kind="ExternalInput")
    qbwT = nc.dram_tensor("qbwT", [QR, 768], BF, kind="ExternalInput")
    kvbwT = nc.dram_tensor("kvbwT", [KR, 1024], BF, kind="ExternalInput")
    owr_d = nc.dram_tensor("owr", [128, 32 * MCOLS], BF, kind="ExternalInput")
    ident_d = nc.dram_tensor("ident", [128, 128], F, kind="ExternalInput")
    triu_d = nc.dram_tensor("triu", [128, 128], BF, kind="ExternalInput")
    sinq_d = nc.dram_tensor("sinq", [128, S], F, kind="ExternalInput")  # sign-folded
    cosq_d = nc.dram_tensor("cosq", [128, S], F, kind="ExternalInput")
    sink_d = nc.dram_tensor("sink", [128, 64], F, kind="ExternalInput")
    cosk_d = nc.dram_tensor("cosk", [128, 64], F, kind="ExternalInput")
    out_d = nc.dram_tensor("out", [MCOLS, S], BF, kind="ExternalOutput")

    g1kv_src = nc.dram_tensor("g1kv_src", [KR + DR, SL], BF)
    g1kv = nc.dram_tensor("g1kv", [NCORES, KR + DR, SL], BF, addr_space="Shared")
    g1q_src = nc.dram_tensor("g1q_src", [QR, SL], BF)
    g1q = nc.dram_tensor("g1q", [NCORES, QR, SL], BF, addr_space="Shared")
    g2_src = [nc.dram_tensor(f"g2src{h}", [DV, S], BF) for h in range(HPC)]
    g2 = [
        nc.dram_tensor(f"g2_{h}", [NCORES, DV, S], BF, addr_space="Shared")
        for h in range(HPC)
    ]
    g2_src3 = [nc.dram_tensor(f"g2src3{i}", [DV, S // 2], BF) for i in range(2)]
    g2h3 = [
        nc.dram_tensor(f"g2h3{i}", [NCORES, DV, S // 2], BF, addr_space="Shared")
        for i in range(2)
    ]
    RG = [list(range(NCORES))]
    NKT = H // 128  # 32 k-tiles over the model dim
    NR = QR // 128  # 12 k-tiles over q_lora_rank
    NKR = KR // 128  # 4 k-tiles over kv_lora_rank

    with TileContext(nc) as tc, ExitStack() as ctx:
        persist = ctx.enter_context(tc.tile_pool(name="persist", bufs=1))

        # ---------------- constants ----------------
        ident = persist.tile([128, 128], F, name="c_ident")
        nc.sync.dma_start(ident[:], ident_d[:])
        triu = persist.tile([128, 128], BF, name="c_triu")
        nc.sync.dma_start(triu[:], triu_d[:])
        sin_k2 = persist.tile([128, 64], F, name="c_sink")
        nc.sync.dma_start(sin_k2[:], sink_d[:])
        cos_k2 = persist.tile([128, 64], F, name="c_cosk")
        nc.sync.dma_start(cos_k2[:], cosk_d[:])
        ones_mat = persist.tile([128, 128], BF, name="c_ones_mat")
        nc.vector.memset(ones_mat[:], 1.0)
        ones_row = persist.tile([1, 128], BF, name="c_ones_row")
        nc.vector.memset(ones_row[:], 1.0)
        eps_t = persist.tile([128, 1], F, name="c_eps")
        nc.vector.memset(eps_t[:], EPS)
        identb = persist.tile([128, 128], BF, name="c_identb")
        nc.vector.tensor_copy(identb[:], ident[:])
        sin_k = [sin_k2[:, ts(st, 32)] for st in range(2)]
        cos_k = [cos_k2[:, ts(st, 32)] for st in range(2)]

        # ---------------- phase 0: resident x + weights ----------------
        ctx0 = ExitStack()
        xtp = ctx0.enter_context(tc.tile_pool(name="xtp", bufs=1))
        p0 = ctx0.enter_context(tc.tile_pool(name="p0", bufs=2))

        xt = xtp.tile([128, NKT, SL], BF, name="xt")
        kvw = [
            xtp.tile([128, 8, 576], BF, tag=f"kvw{cg}", name=f"kvw{cg}")
            for cg in range(4)
        ]
        # interleave x / kv_a weight chunks so the first matmuls start early
        for cg in range(4):
            nc.sync.dma_start(
                xt[:, ds(8 * cg, 8)],
                xr_d.ap()[:, ds(cg * 8 * SL, 8 * SL)]
                .rearrange("p (kt s) -> p kt s", kt=8),
            )
            nc.sync.dma_start(
                kvw[cg][:],
                kvawr_d.ap()[:, ds(cg * 8 * 576, 8 * 576)]
                .rearrange("p (kt n) -> p kt n", kt=8),
            )
        # q_a weights: 4 chunks of 8 k-tiles
        qw = []
        for cg in range(4):
            t = xtp.tile([128, 8, QR], BF, tag=f"qw{cg}", name=f"qw{cg}")
            nc.sync.dma_start(
                t[:],
                qawr_d.ap()[:, ds(cg * 8 * QR, 8 * QR)]
                .rearrange("p (kt n) -> p kt n", kt=8),
            )
            qw.append(t)

        # ---------------- kv_a projection ----------------
        ctx0a = ExitStack()
        kv_ps = ctx0a.enter_context(tc.tile_pool(name="kv_ps", bufs=4, space="PSUM"))
        kvch = [[None] * 2 for _ in range(2)]
        for ch in range(2):
            for st in range(2):
                kvch[st][ch] = kv_ps.tile(
                    [128, 288], F, tag="kv_ps", name=f"kvps{st}_{ch}"
                )
        for kt in range(NKT):
            w = kvw[kt // 8]
            for ch in range(2):
                for st in range(2):
                    nc.tensor.matmul(
                        kvch[st][ch][:], xt[:, kt, ts(st, 128)],
                        w[:, kt % 8, ts(ch, 288)],
                        start=(kt == 0), stop=(kt == NKT - 1),
                    )
        # copy kv PSUM to SBUF so the q_a PSUM pool can open immediately
        kvsb = [[None] * 2 for _ in range(2)]
        for st in range(2):
            for ch in range(2):
                t = p0.tile([128, 288], F, tag="kvsb", name=f"kvsb{st}_{ch}")
                nc.any.tensor_copy(t[:], kvch[st][ch][:])
                kvsb[st][ch] = t
        ctx0a.close()

        ctx0b = ExitStack()
        qa_ps = ctx0b.enter_context(tc.tile_pool(name="qa_ps", bufs=6, space="PSUM"))
        trp = ctx0b.enter_context(tc.tile_pool(name="trp", bufs=2, space="PSUM"))
        trsb = ctx0b.enter_context(tc.tile_pool(name="trsb", bufs=6))

        # kv tail compute (ACT/Vec only; runs while q_a st0 matmuls occupy PE)
        ckvns, kr_ts = [], []
        for st in range(2):
            acc0 = p0.tile([128, 1], F, tag="kvacc", name=f"kvacc0_{st}")
            acc1 = p0.tile([128, 1], F, tag="kvacc", name=f"kvacc1_{st}")
            scr = p0.tile([128, 288], F, tag="kvscr", name=f"kvscr{st}")
            nc.scalar.activation(scr[:], kvsb[st][0][:], AF.Square, accum_out=acc0[:])
            nc.scalar.activation(
                scr[:, 0:224], kvsb[st][1][:, 0:224], AF.Square, accum_out=acc1[:]
            )
            nc.vector.tensor_tensor(acc0[:], acc0[:], acc1[:], ALU.add)
            stdv = p0.tile([128, 1], F, tag="kvstd", name=f"kvstd{st}")
            nc.scalar.activation(stdv[:], acc0[:], AF.Sqrt, bias=eps_t[:], scale=1.0 / KR)
            rinv = p0.tile([128, 1], F, tag="kvrinv", name=f"kvrinv{st}")
            nc.vector.reciprocal(rinv[:], stdv[:])
            ckvn = p0.tile([128, KR], BF, tag="ckvn", name=f"ckvn{st}")
            nc.vector.tensor_scalar_mul(ckvn[:, 0:288], kvsb[st][0][:], rinv[:])
            nc.vector.tensor_scalar_mul(ckvn[:, 288:512], kvsb[st][1][:, 0:224], rinv[:])
            # rope k_pe: cols 512:576 of kv_a = chunk1 cols 224:288, deinterleaved
            pe = kvsb[st][1][:, 224:288].rearrange("p (d two) -> p two d", two=2)
            y1, y2 = pe[:, 0], pe[:, 1]
            kr_t = p0.tile([128, DR], BF, tag="kr", name=f"kr{st}")
            t1 = p0.tile([128, 32], F, tag="krt1", name=f"krt1_{st}")
            t2 = p0.tile([128, 32], F, tag="krt2", name=f"krt2_{st}")
            nc.vector.tensor_tensor(t1[:], y1, cos_k[st][:], ALU.mult)
            nc.vector.tensor_tensor(t2[:], y2, sin_k[st][:], ALU.mult)
            nc.vector.tensor_tensor(kr_t[:, 0:32], t1[:], t2[:], ALU.subtract)
            nc.vector.tensor_tensor(t1[:], y2, cos_k[st][:], ALU.mult)
            nc.vector.tensor_tensor(t2[:], y1, sin_k[st][:], ALU.mult)
            nc.vector.tensor_tensor(kr_t[:, 32:64], t1[:], t2[:], ALU.add)
            ckvns.append(ckvn)
            kr_ts.append(kr_t)

        qch = [[None] * 3 for _ in range(2)]
        for ch in range(3):
            for st in range(2):
                qch[st][ch] = qa_ps.tile(
                    [128, 512], F, tag="qa_ps", name=f"qaps{st}_{ch}"
                )

        def q_mms(st, kt_lo=0, kt_hi=NKT):
            for kt in range(kt_lo, kt_hi):
                w = qw[kt // 8]
                for ch in range(3):
                    nc.tensor.matmul(
                        qch[st][ch][:], xt[:, kt, ts(st, 128)],
                        w[:, kt % 8, ts(ch, 512)],
                        start=(kt == 0), stop=(kt == NKT - 1),
                    )

        q_mms(0, 0, 8)  # ~7us of PE work: covers the kv rmsnorm latency

        # kv transposes slot into the PE stream here (rmsnorm long done)
        for st in range(2):
            for rt in range(KR // 128):
                tp = trp.tile([128, 128], BF, tag="tr", name=f"kvtr{st}_{rt}")
                nc.tensor.transpose(tp[:], ckvns[st][:, ts(rt, 128)], identb[:])
                sb_t = trsb.tile([128, 128], BF, tag="trsb", name=f"kvtrs{st}_{rt}")
                nc.any.tensor_copy(sb_t[:], tp[:])
                nc.gpsimd.dma_start(g1kv_src.ap()[ts(rt, 128), ts(st, 128)], sb_t[:])
            tpk = trp.tile([64, 128], BF, tag="tr", name=f"kvtrk{st}")
            nc.tensor.transpose(tpk[:], kr_ts[st][:], identb[:])
            sb_k = trsb.tile([64, 128], BF, tag="trsb", name=f"kvtrks{st}")
            nc.any.tensor_copy(sb_k[:], tpk[:])
            nc.gpsimd.dma_start(g1kv_src.ap()[KR : KR + DR, ts(st, 128)], sb_k[:])

        nc.gpsimd.collective_compute(
            "AllGather", ALU.bypass,
            ins=[g1kv_src.ap().opt()], outs=[g1kv.ap().opt()], replica_groups=RG,
        )

        def q_tail_rms(st):
            accs = []
            scr = p0.tile([128, 512], F, tag="qascr", name=f"qascr{st}")
            for ch in range(3):
                a = p0.tile([128, 1], F, tag="qaacc", name=f"qaacc{st}_{ch}")
                nc.scalar.activation(scr[:], qch[st][ch][:], AF.Square, accum_out=a[:])
                accs.append(a)
            nc.vector.tensor_tensor(accs[0][:], accs[0][:], accs[1][:], ALU.add)
            nc.vector.tensor_tensor(accs[0][:], accs[0][:], accs[2][:], ALU.add)
            stdv = p0.tile([128, 1], F, tag="qastd", name=f"qastd{st}")
            nc.scalar.activation(stdv[:], accs[0][:], AF.Sqrt, bias=eps_t[:], scale=1.0 / QR)
            rinv = p0.tile([128, 1], F, tag="qarinv", name=f"qarinv{st}")
            nc.vector.reciprocal(rinv[:], stdv[:])
            qn = p0.tile([128, QR], BF, tag="qn", name=f"qn{st}")
            for ch in range(3):
                nc.vector.tensor_scalar_mul(qn[:, ts(ch, 512)], qch[st][ch][:], rinv[:])
            return qn

        def q_tail_tr(st, qn):
            for rt in range(NR):
                tp = trp.tile([128, 128], BF, tag="tr", name=f"qtr{st}_{rt}")
                nc.tensor.transpose(tp[:], qn[:, ts(rt, 128)], identb[:])
                sb_t = trsb.tile([128, 128], BF, tag="trsb", name=f"qtrs{st}_{rt}")
                nc.any.tensor_copy(sb_t[:], tp[:])
                nc.gpsimd.dma_start(g1q_src.ap()[ts(rt, 128), ts(st, 128)], sb_t[:])

        q_mms(0, 8, NKT)
        qn0 = q_tail_rms(0)  # ACT/Vec: overlaps st1 matmuls
        q_mms(1)
        q_tail_tr(0, qn0)
        qn1 = q_tail_rms(1)
        q_tail_tr(1, qn1)
        ctx0b.close()
        ctx0.close()  # frees x + weights SBUF

        nc.gpsimd.collective_compute(
            "AllGather", ALU.bypass,
            ins=[g1q_src.ap().opt()], outs=[g1q.ap().opt()], replica_groups=RG,
        )

        # ---------------- shared latents on-chip ----------------
        ctx_att = ExitStack()
        attb = ctx_att.enter_context(tc.tile_pool(name="attb", bufs=1))

        kpe_rep = attb.tile([128, S], BF, name="kpe_rep")
        for half in range(2):
            nc.sync.dma_start(
                kpe_rep[ds(64 * half, 64), :].rearrange("p (c s) -> p c s", c=NCORES),
                g1kv.ap()[:, KR : KR + DR, :].rearrange("c p s -> p c s"),
            )

        qnope = [attb.tile([128, S], BF, name=f"qnope{h}") for h in range(HPC)]
        qfpe = [attb.tile([128, S], BF, name=f"qfpe{p}") for p in range(2)]
        v_sb = attb.tile([128, S // 128, 512], BF, name="v_sb")
        kn_all = [attb.tile([128, S], BF, name=f"kn{h}") for h in range(HPC)]

        # q rope tables from host (sin_q/cos_q over full S)
        ctx_sin = ExitStack()  # sin_q/cos_q live until after q_b
        sinp = ctx_sin.enter_context(tc.tile_pool(name="sinp", bufs=1))
        sin_q = sinp.tile([128, S], F, name="t_sinq")
        nc.sync.dma_start(sin_q[:], sinq_d[:])
        cos_q = sinp.tile([128, S], F, name="t_cosq")
        nc.sync.dma_start(cos_q[:], cosq_d[:])

        # ---------------- kv_b decompression (before q_b: overlaps AG2) ----
        ctxd = ExitStack()
        ckvp = ctxd.enter_context(tc.tile_pool(name="ckvp", bufs=1))
        dec_ps = ctxd.enter_context(tc.tile_pool(name="dec_ps", bufs=2, space="PSUM"))
        ckv_t = []
        for r in range(NKR):
            t = ckvp.tile([128, S], BF, name=f"ckv{r}")
            nc.sync.dma_start(
                t[:].rearrange("p (c s) -> p c s", c=NCORES),
                g1kv.ap()[:, ts(r, 128), :].rearrange("c p s -> p c s"),
            )
            ckv_t.append(t)
        kvbv, kvbn = [], []
        for r in range(NKR):
            tv = ckvp.tile([128, 512], BF, name=f"kvbv{r}")
            nc.sync.dma_start(tv[:], kvbwT.ap()[ts(r, 128), 512:1024])
            kvbv.append(tv)
            tn = ckvp.tile([128, 512], BF, name=f"kvbn{r}")
            nc.sync.dma_start(tn[:], kvbwT.ap()[ts(r, 128), 0:512])
            kvbn.append(tn)

        for st in range(S // 128):
            ps = dec_ps.tile([128, 512], F, tag="dec", name=f"vps{st}")
            for r in range(NKR):
                nc.tensor.matmul(
                    ps[:], ckv_t[r][:, ts(st, 128)], kvbv[r][:],
                    start=(r == 0), stop=(r == NKR - 1),
                )
            nc.any.tensor_copy(v_sb[:, st, :], ps[:])
        for h in range(HPC):
            for sb in range(4):
                ps = dec_ps.tile([128, 512], F, tag="dec", name=f"knps{h}_{sb}")
                for r in range(NKR):
                    nc.tensor.matmul(
                        ps[:], kvbn[r][:, ts(h, 128)], ckv_t[r][:, ts(sb, 512)],
                        start=(r == 0), stop=(r == NKR - 1),
                    )
                nc.any.tensor_copy(kn_all[h][:, ts(sb, 512)], ps[:])
        ctxd.close()

        # ---------------- q_b projection (+ q rope) ----------------
        ctx2 = ExitStack()
        qrp = ctx2.enter_context(tc.tile_pool(name="qrp", bufs=24))
        qbwp = ctx2.enter_context(tc.tile_pool(name="qbwp", bufs=1))
        ropep = ctx2.enter_context(tc.tile_pool(name="ropep", bufs=2))
        qb_ps = ctx2.enter_context(tc.tile_pool(name="qb_ps", bufs=6, space="PSUM"))

        qbw_all = qbwp.tile([128, NR, 768], BF, name="qbw_all")
        nc.sync.dma_start(
            qbw_all[:], qbwT.ap().rearrange("(r p) n -> p r n", p=128)
        )
        for sb in range(4):
            qr_tiles = []
            for r in range(NR):
                t = qrp.tile([128, 512], BF, tag="qr", name=f"qr{sb}_{r}")
                nc.sync.dma_start(
                    t[:].rearrange("p (c s) -> p c s", c=2),
                    g1q.ap()[2 * sb : 2 * sb + 2, ts(r, 128), :]
                    .rearrange("c p s -> p c s"),
                )
                qr_tiles.append(t)
            psj = [
                qb_ps.tile([128, 512], F, tag="qb_ps", name=f"qbps{sb}_{j}")
                for j in range(6)
            ]
            for r in range(NR):
                for j in range(6):
                    nc.tensor.matmul(
                        psj[j][:], qbw_all[:, r, ts(j, 128)], qr_tiles[r][:],
                        start=(r == 0), stop=(r == NR - 1),
                    )
            for h in range(HPC):
                nc.any.tensor_copy(qnope[h][:, ts(sb, 512)], psj[h][:])
            for p in range(2):
                t1 = ropep.tile([128, 512], F, tag="rope1", name=f"rp1_{sb}_{p}")
                t2 = ropep.tile([128, 512], F, tag="rope2", name=f"rp2_{sb}_{p}")
                nc.vector.tensor_tensor(
                    t1[:], psj[4 + p][:], cos_q[:, ts(sb, 512)], ALU.mult
                )
                for strip in range(4):
                    s0 = 32 * strip
                    nc.vector.tensor_tensor(
                        t2[ds(s0, 32), :],
                        psj[4 + p][ds(s0 ^ 32, 32), :],
                        sin_q[ds(s0, 32), ts(sb, 512)],
                        ALU.mult,
                    )
                nc.vector.tensor_tensor(qfpe[p][:, ts(sb, 512)], t1[:], t2[:], ALU.add)
        ctx2.close()
        ctx_sin.close()

        # ---------------- attention + interleaved o_proj ----------------
        ctx3 = ExitStack()
        probp = ctx3.enter_context(tc.tile_pool(name="probp", bufs=4))
        attp = ctx3.enter_context(tc.tile_pool(name="attp", bufs=3))
        denp = ctx3.enter_context(tc.tile_pool(name="denp", bufs=2))
        sc_ps = ctx3.enter_context(tc.tile_pool(name="sc_ps", bufs=3, space="PSUM"))
        av_ps = ctx3.enter_context(tc.tile_pool(name="av_ps", bufs=2, space="PSUM"))
        den_ps = ctx3.enter_context(tc.tile_pool(name="den_ps", bufs=2, space="PSUM"))
        bc_ps = ctx3.enter_context(tc.tile_pool(name="bc_ps", bufs=1, space="PSUM"))

        # o_proj resources (interleaved with attention)
        owp = ctx3.enter_context(tc.tile_pool(name="owp", bufs=1))
        g2p = ctx3.enter_context(tc.tile_pool(name="g2p", bufs=10))
        osbp = ctx3.enter_context(tc.tile_pool(name="osbp", bufs=1))
        ow_all = owp.tile([128, 32, MCOLS], BF, name="ow_all")
        nc.sync.dma_start(
            ow_all[:], owr_d.ap().rearrange("p (b m) -> p b m", b=32)
        )
        osb = [
            [osbp.tile([128, 512], F, name=f"osb{mt}_{sblk}") for sblk in range(4)]
            for mt in range(4)
        ]

        def oproj_block(g, half=None):
            g2rows = []
            for r8 in range(NCORES):
                if g == HPC - 1:
                    gt = g2p.tile(
                        [128, S // 2], BF, tag="g2th", name=f"g2t{g}_{half}_{r8}",
                        bufs=8,
                    )
                    nc.sync.dma_start(gt[:], g2h3[half].ap()[r8, :, :])
                else:
                    gt = g2p.tile([128, S], BF, tag="g2t", name=f"g2t{g}_{r8}")
                    nc.sync.dma_start(gt[:], g2[g].ap()[r8, :, :])
                g2rows.append(gt)
            sblks = range(4) if g < HPC - 1 else range(2 * half, 2 * half + 2)
            for sblk in sblks:
                loc = sblk if g < HPC - 1 else sblk - 2 * half
                for mt in range(4):
                    ps = sc_ps.tile([128, 512], F, tag="sc", name=f"ops{g}_{sblk}_{mt}")
                    for r8 in range(NCORES):
                        nc.tensor.matmul(
                            ps[:], ow_all[:, HPC * r8 + g, ts(mt, 128)],
                            g2rows[r8][:, ts(loc, 512)],
                            start=(r8 == 0), stop=(r8 == NCORES - 1),
                        )
                    if g == 0:
                        nc.any.tensor_copy(osb[mt][sblk][:], ps[:])
                    elif g < HPC - 1:
                        nc.vector.tensor_tensor(
                            osb[mt][sblk][:], osb[mt][sblk][:], ps[:], ALU.add
                        )
                    else:
                        obf = attp.tile(
                            [128, 512], BF, tag="obf", name=f"obf{mt}_{sblk}"
                        )
                        nc.vector.tensor_tensor(
                            obf[:], osb[mt][sblk][:], ps[:], ALU.add
                        )
                        nc.gpsimd.dma_start(
                            out_d.ap()[ts(mt, 128), ts(sblk, 512)], obf[:]
                        )

        # deferred softmax normalize: bc matmul + mult emitted after the next
        # qb's first score matmuls so the PE never waits on the Vec reciprocal
        pending_norm = [None]

        def flush_norm():
            if pending_norm[0] is None:
                return
            avt_p, den_sb_p, h_p, qb_p = pending_norm[0]
            pending_norm[0] = None
            bc = bc_ps.tile([128, 512], F, tag="bc", name=f"bc{h_p}_{qb_p}")
            nc.tensor.matmul(bc[:], ones_row[:], den_sb_p[:], start=True, stop=True)
            bcs = denp.tile([128, 512], BF, tag="bcs", name=f"bcs{h_p}_{qb_p}")
            nc.scalar.activation(bcs[:], bc[:], AF.Copy)
            att = attp.tile([128, 512], BF, tag="att", name=f"att{h_p}_{qb_p}")
            nc.vector.tensor_tensor(att[:], avt_p[:], bcs[:], ALU.mult)
            if h_p == HPC - 1:
                nc.gpsimd.dma_start(
                    g2_src3[qb_p // 2].ap()[:, ts(qb_p % 2, 512)], att[:]
                )
            else:
                nc.gpsimd.dma_start(g2_src[h_p].ap()[:, ts(qb_p, 512)], att[:])

        for h in range(HPC):
            kn = kn_all[h]
            pe_rhs = qfpe[h // 2][ds(64 * (h % 2), 64), :]
            pe_lhs = kpe_rep[ds(64 * (h % 2), 64), :]
            for qb in range(4):
                nkt = 4 * (qb + 1)
                avt = av_ps.tile([128, 512], F, tag="av", name=f"av{h}_{qb}")
                dent = den_ps.tile([128, 512], F, tag="den", name=f"den{h}_{qb}")

                def av_den(pprob, pkt, ptrim, last):
                    nc.tensor.matmul(
                        avt[:, ptrim:512], v_sb[:, pkt, ts(h, 128)],
                        pprob[:, ptrim:512],
                        start=(pkt == 0), stop=last,
                    )
                    nc.tensor.matmul(
                        dent[:, ptrim:512], ones_mat[:], pprob[:, ptrim:512],
                        start=(pkt == 0), stop=last,
                    )

                pend = []  # (prob, kt, trim), depth-2 software pipeline
                for kt in range(nkt):
                    trim = max(0, 128 * (kt - 4 * qb))
                    qsl = ds(512 * qb + trim, 512 - trim)
                    sct = sc_ps.tile([128, 512], F, tag="sc", name=f"sc{h}{qb}_{kt}")
                    nc.tensor.matmul(
                        sct[:, trim:512], kn[:, ts(kt, 128)], qnope[h][:, qsl],
                        start=True, stop=False,
                    )
                    nc.tensor.matmul(
                        sct[:, trim:512], pe_lhs[:, ts(kt, 128)], pe_rhs[:, qsl],
                        start=False, stop=True,
                    )
                    if kt == 2:
                        flush_norm()
                    if len(pend) == 2:
                        av_den(*pend.pop(0), last=False)
                    prob = probp.tile([128, 512], BF, tag="prob", name=f"pr{h}{qb}_{kt}")
                    nc.scalar.activation(prob[:, trim:512], sct[:, trim:512], AF.Exp)
                    if kt >= 4 * qb:
                        nc.vector.tensor_tensor(
                            prob[:, trim : trim + 128],
                            prob[:, trim : trim + 128],
                            triu[:],
                            ALU.mult,
                        )
                    pend.append((prob, kt, trim))
                while pend:
                    av_den(*pend.pop(0), last=(len(pend) == 0))
                # 1/den via bit-trick seed + one Newton step on DVE (the DVE
                # RECIPROCAL op has a ~3us fixed cost; ACT Ln would thrash
                # activation tables against Exp). |rel err| < 0.1% + bf16.
                y0i = denp.tile([1, 512], I32, tag="y0i", name=f"y0i{h}_{qb}")
                nc.vector.tensor_scalar(
                    y0i[:], dent[0:1, :].bitcast(I32), -1, None, ALU.bitwise_xor
                )
                nc.vector.tensor_scalar(y0i[:], y0i[:], 0x7EF311C4, None, ALU.add)
                nt = denp.tile([1, 512], F, tag="nt", name=f"nt{h}_{qb}")
                nc.vector.tensor_tensor(
                    nt[:], dent[0:1, :], y0i[:].bitcast(F), ALU.mult
                )
                nc.vector.tensor_scalar(nt[:], nt[:], -1.0, 2.0, ALU.mult, ALU.add)
                den_sb = denp.tile([1, 512], BF, tag="densb", name=f"densb{h}_{qb}")
                nc.vector.tensor_tensor(
                    den_sb[:], y0i[:].bitcast(F), nt[:], ALU.mult
                )
                pending_norm[0] = (avt, den_sb, h, qb)
                if h == HPC - 1 and qb == 1:
                    # first half of the last head gathers early so its o_proj
                    # half can run as soon as the head finishes
                    flush_norm()
                    nc.gpsimd.collective_compute(
                        "AllGather", ALU.bypass,
                        ins=[g2_src3[0].ap().opt()], outs=[g2h3[0].ap().opt()],
                        replica_groups=RG,
                    )

            flush_norm()  # last qb's normalize must land before the AllGather
            if h < HPC - 1:
                nc.gpsimd.collective_compute(
                    "AllGather", ALU.bypass,
                    ins=[g2_src[h].ap().opt()], outs=[g2[h].ap().opt()],
                    replica_groups=RG,
                )
                if h > 0:
                    oproj_block(h - 1)
            else:
                nc.gpsimd.collective_compute(
                    "AllGather", ALU.bypass,
                    ins=[g2_src3[1].ap().opt()], outs=[g2h3[1].ap().opt()],
                    replica_groups=RG,
                )
                oproj_block(HPC - 2)
                oproj_block(HPC - 1, half=0)
                oproj_block(HPC - 1, half=1)
        ctx3.close()
        ctx_att.close()

    nc.compile()
    return nc


def _get_nc():
    global _CACHED_NC
    if _CACHED_NC is None:
        _CACHED_NC = _build_program()
    return _CACHED_NC


def _prep_inputs(hidden_states, position_ids, q_a_w, q_a_ln_w, q_b_w, kv_a_w,
                 kv_a_ln_w, kv_b_w, o_w):
    hidden_states = np.asarray(hidden_states, dtype=np.float32)
    position_ids = np.asarray(position_ids, dtype=np.int32)
    q_a_w = np.asarray(q_a_w, dtype=np.float32)
    q_a_ln_w = np.asarray(q_a_ln_w, dtype=np.float32)
    q_b_w = np.asarray(q_b_w, dtype=np.float32)
    kv_a_w = np.asarray(kv_a_w, dtype=np.float32)
    kv_a_ln_w = np.asarray(kv_a_ln_w, dtype=np.float32)
    kv_b_w = np.asarray(kv_b_w, dtype=np.float32)
    o_w = np.asarray(o_w, dtype=np.float32)

    bf = ml_dtypes.bfloat16
    xT_full = np.ascontiguousarray(hidden_states[0].T)          # (H, S)
    # [H=(kt p), n] -> [p, kt*n] prearrangement for contiguous DMA
    qawT = q_a_w.T.reshape(32, 128, QR).transpose(1, 0, 2).reshape(128, 32 * QR)
    kvawT = kv_a_w.T.reshape(32, 128, 576).transpose(1, 0, 2).reshape(128, 32 * 576)
    ident = np.eye(128, dtype=np.float32)
    triu = np.triu(np.ones((128, 128), dtype=np.float32))        # keep k <= q
    inv_freq = (1.0 / (THETA ** (np.arange(0, DR, 2) / DR))).astype(np.float32)
    if128 = np.tile(inv_freq, 4)                                 # (128,)
    posf = position_ids.reshape(-1).astype(np.float64)           # (S,)
    ang_q = if128[:, None] * posf[None, :]                       # (128, S)
    # sign-folded sin for the strip-shifted rope:
    # qfpe[d] = pe_d[d]*cos[d] + pe_d[d^32]*sgn[d]*sin[d], sgn=-1 for d%64<32
    sgn = np.where((np.arange(128) % 64) < 32, -1.0, 1.0)[:, None]
    sinq = (sgn * np.sin(ang_q)).astype(np.float32)
    cosq = np.cos(ang_q).astype(np.float32)
    owT_full = np.ascontiguousarray(o_w.T)                       # (N*DV, H)

    in_maps = []
    for c in range(NCORES):
        heads = slice(HPC * c, HPC * (c + 1))
        qb = q_b_w.reshape(N_HEADS, QD, QR)[heads]               # (4, 192, QR)
        nope = qb[:, :DN, :].reshape(HPC * DN, QR)
        pe = qb[:, DN:, :]
        pe_d = np.concatenate([pe[:, 0::2, :], pe[:, 1::2, :]], axis=1)  # (4,64,QR)
        cols = np.concatenate(
            [nope, pe_d.reshape(HPC * DR, QR)], axis=0
        )                                                        # (768, QR)
        qbwT_c = np.ascontiguousarray((cols * (SCALE * q_a_ln_w[None, :])).T)

        kvb = kv_b_w.reshape(N_HEADS, DN + DV, KR)[heads]
        kcols = np.concatenate(
            [kvb[:, :DN, :].reshape(HPC * DN, KR),
             kvb[:, DN:, :].reshape(HPC * DV, KR)],
            axis=0,
        )                                                        # (1024, KR)
        kvbwT_c = np.ascontiguousarray((kcols * kv_a_ln_w[None, :]).T)

        ow_slice = owT_full[:, MCOLS * c : MCOLS * (c + 1)]      # (4096, 512)
        owr = ow_slice.reshape(32, 128, MCOLS).transpose(1, 0, 2).reshape(
            128, 32 * MCOLS
        )

        x_slice = xT_full[:, SL * c : SL * (c + 1)]              # (H, SL)
        xr = x_slice.reshape(32, 128, SL).transpose(1, 0, 2).reshape(128, 32 * SL)

        pos_loc = posf[SL * c : SL * (c + 1)]                    # (256,)
        ang_k = pos_loc.reshape(2, 128).T[:, :, None] * inv_freq[None, None, :]
        sink = np.sin(ang_k).reshape(128, 64).astype(np.float32)  # (128, 2*32)
        cosk = np.cos(ang_k).reshape(128, 64).astype(np.float32)

        in_maps.append(
            {
                "xr": np.ascontiguousarray(xr).astype(bf),
                "qawr": np.ascontiguousarray(qawT).astype(bf),
                "kvawr": np.ascontiguousarray(kvawT).astype(bf),
                "qbwT": qbwT_c.astype(bf),
                "kvbwT": kvbwT_c.astype(bf),
                "owr": np.ascontiguousarray(owr).astype(bf),
                "ident": ident,
                "triu": triu.astype(bf),
                "sinq": sinq,
                "cosq": cosq,
                "sink": np.ascontiguousarray(sink),
                "cosk": np.ascontiguousarray(cosk),
            }
        )
    return in_maps


def kernel(**inputs):
    global LAST_RESULT
    nc = _get_nc()
    in_maps = _prep_inputs(**inputs)
    res = run_bass_kernel_spmd(nc, in_maps, list(range(NCORES)))
    LAST_RESULT = res
    out = np.concatenate([res.results[c]["out"].T for c in range(NCORES)], axis=1)
    return out[None].astype(np.float32)
